# revision 22
# baseline (speedup 1.0000x reference)
"""Trainium2 Bass kernel for nn_Attention (B=8, Sq=Skv=2048, d=512).

Sharding: data-parallel over batch -- core b handles batch b (8 cores).

Per-core pipeline (one NeuronCore, Tile-scheduled):
  stage 1 (per 128-row tile of ques/keys/vals):
    DMA in -> PE-transpose blocks (batched into one PSUM bank) -> fp32r
    projection matmuls -> layernorm rows for q/k (bn_stats on DVE,
    rstd = exp(-0.5*ln(var+eps)) on ACT so the only act tables used
    anywhere are ln/exp/copy/identity -- one table load total) ->
    PE-transpose q/k into [d, seq] fp32r layout with the ln gain/bias
    (and 1/sqrt(dk) for q) fused into the PSUM eviction.
  stage 2 (per 128-row query tile t; causal: kv <= 128(t+1)):
    S chunks = qT.T @ kT (fp32r) -> triangular mask on the diagonal
    block (+ key mask only if any key is masked -- compile-time
    specialization) -> exp on ACT with fused row-sum accumulation (no
    max subtraction: |S| <= sqrt(dk)*max|g|^2 since q/k are
    layernormed, well inside fp32 range) -> PE-transpose P blocks
    (batched) -> PV fp32r accumulation -> fused (o/rowsum + residual)
    on DVE -> output layernorm (gpsimd applies gain/bias) -> DMA out.

Engine routing is explicit: ACT owns exp/ln + PSUM evictions (copy),
DVE owns fused scale/shift + stats + masks, GPSIMD owns the output
gain/bias elementwise passes, PE owns matmuls/transposes.
"""

import math
import numpy as np

B = 8
S = 2048
D = 512
P = 128
KC = D // P       # 4 feature chunks
NT = S // P       # 16 seq tiles
EPS = 1e-5
NEG = np.float32(-1e30)

_CACHE = {}


def _round_f32r(a):
    """Round fp32 to the PE's f32r grid: RNE keeping 11 mantissa bits
    (measured on hardware: low 12 mantissa bits dropped, ties-to-even)."""
    b = np.ascontiguousarray(a, np.float32).view(np.uint32).astype(np.int64)
    low = b & 0xFFF
    base = b & ~np.int64(0xFFF)
    up = base + 0x1000
    r = np.where(low > 0x800, up,
                 np.where(low < 0x800, base,
                          np.where((base >> 12) & 1, up, base)))
    return r.astype(np.uint32).view(np.float32).reshape(a.shape)


def _build(has_km):
    from contextlib import ExitStack

    import concourse.bass as bass
    import concourse.tile as tile
    from concourse import bacc, mybir

    f32 = mybir.dt.float32
    f32r = mybir.dt.float32r
    Alu = mybir.AluOpType
    Act = mybir.ActivationFunctionType

    class OneActSetBacc(bacc.Bacc):
        """Force every activation onto the ln+exp+copy+identity table set.

        The default chooser maps each function to the first act-func-set
        containing it (Exp -> set 0, Ln -> set 5), which makes alternating
        ln/exp insert a ~1.3us table load per pair.  This kernel only uses
        functions that all live in 'natural_log_exp_and_others', so empty
        out the earlier sets; the fixpoint pass then emits one load total.
        """

        def insert_act_table_loads(self):
            import bass_rust as _bass_rust
            from concourse.hw_specs import get_activation_tables

            has_activation = any(
                isinstance(i, mybir.InstActivation)
                for b in self.main_func.blocks
                for i in b.instructions
            )
            if not has_activation:
                return
            tables = list(get_activation_tables(self.m.arch).items())
            target = next(i for i, (n, _) in enumerate(tables)
                          if n == "natural_log_exp_and_others")
            tables = [(n, (s if i >= target else set()))
                      for i, (n, s) in enumerate(tables)]
            _bass_rust.insert_act_table_loads(self, tables)

    nc = OneActSetBacc("TRN2", target_bir_lowering=False, debug=False,
                       num_devices=B)

    xq_d = nc.dram_tensor("xq", [S, D], f32, kind="ExternalInput").ap()
    xqr_d = nc.dram_tensor("xqr", [S, D], f32r, kind="ExternalInput").ap()
    xk_d = nc.dram_tensor("xk", [S, D], f32r, kind="ExternalInput").ap()
    xv_d = nc.dram_tensor("xv", [S, D], f32r, kind="ExternalInput").ap()
    wq_d = nc.dram_tensor("wq", [D, D], f32r, kind="ExternalInput").ap()
    wk_d = nc.dram_tensor("wk", [D, D], f32r, kind="ExternalInput").ap()
    wv_d = nc.dram_tensor("wv", [D, D], f32r, kind="ExternalInput").ap()
    gbv_d = nc.dram_tensor("gbv", [P, 4 * KC], f32, kind="ExternalInput").ap()
    gobo_d = nc.dram_tensor("gobo", [P, 2 * D], f32, kind="ExternalInput").ap()
    identr_d = nc.dram_tensor("identr", [P, P], f32r, kind="ExternalInput").ap()
    tri_d = nc.dram_tensor("tri", [P, P], f32, kind="ExternalInput").ap()
    km_d = nc.dram_tensor("km", [P, S], f32, kind="ExternalInput").ap()
    out_d = nc.dram_tensor("out", [S, D], f32, kind="ExternalOutput").ap()

    with tile.TileContext(nc) as tc, ExitStack() as ctx:
        cpool = ctx.enter_context(tc.tile_pool(name="consts", bufs=1))
        wstage = ctx.enter_context(tc.tile_pool(name="wstage", bufs=2))
        xstage = ctx.enter_context(tc.tile_pool(name="xstage", bufs=2))
        xt_pool = ctx.enter_context(tc.tile_pool(name="xt", bufs=3))
        y_pool = ctx.enter_context(tc.tile_pool(name="ypool", bufs=3))
        small = ctx.enter_context(tc.tile_pool(name="small", bufs=6))
        p_pool = ctx.enter_context(tc.tile_pool(name="ppool", bufs=2))
        pt_pool = ctx.enter_context(tc.tile_pool(name="ptpool", bufs=4))
        z_pool = ctx.enter_context(tc.tile_pool(name="zpool", bufs=3))
        big = ctx.enter_context(tc.tile_pool(name="big", bufs=1))

        # ---- constants (packed into few DMAs; host pre-rounds f32r data) ----
        identr_t = cpool.tile([P, P], f32r)
        nc.sync.dma_start(identr_t[:], identr_d)
        ident_r = identr_t[:]
        tri_t = cpool.tile([P, P], f32)
        nc.sync.dma_start(tri_t[:], tri_d)
        tri = tri_t[:]
        gbv = cpool.tile([P, 4 * KC], f32)
        nc.sync.dma_start(gbv[:], gbv_d)
        gb = {nm: gbv[:, j * KC:(j + 1) * KC]
              for j, nm in enumerate(("gq", "bq", "gk", "bk"))}
        eps_sb = cpool.tile([P, 1], f32)
        nc.vector.memset(eps_sb[:], EPS)

        # prefetch the first input tiles so PE transposes can start while
        # the 3MB of weights stream in
        prefetched = {}
        for i in (0, 1):
            for nm, dram in (("xqr", xqr_d), ("xk", xk_d), ("xv", xv_d)):
                xt0 = xstage.tile([P, D], f32r, tag=nm)
                nc.sync.dma_start(xt0[:], dram[i * P:(i + 1) * P, :])
                prefetched[(nm, i)] = xt0

        # weights arrive host-pre-rounded to the f32r grid: direct DMA
        w_r = {}
        for name, dram in (("wq", wq_d), ("wk", wk_d), ("wv", wv_d)):
            wr = cpool.tile([P, KC, D], f32r, tag=name + "r")
            nc.sync.dma_start(wr[:], dram.rearrange("(c p) n -> p c n", p=P))
            w_r[name] = wr

        # stage-2-only constants arrive after the weights
        gobo = cpool.tile([P, 2 * D], f32)
        nc.sync.dma_start(gobo[:], gobo_d)
        go_sb = gobo[:, 0:D]
        bo_sb = gobo[:, D:2 * D]
        if has_km:
            km = cpool.tile([P, S], f32)
            nc.sync.dma_start(km[:], km_d)

        # persistent per-batch tensors
        qT = big.tile([P, KC, S], f32r, tag="qT")       # [d_part, dchunk, seq]
        kT = big.tile([P, KC, S], f32r, tag="kT")
        v_sb = big.tile([P, NT, D], f32r, tag="v")      # [kv_part, kvtile, dv]

        # ---- PSUM pools (tpr shared by both phases; stage-1-only pools
        #      closed before stage 2's S/O pools open) ----
        tpr_ps = ctx.enter_context(tc.tile_pool(name="tpr_ps", bufs=2, space="PSUM"))
        s1ctx = ExitStack()
        tp1_ps = s1ctx.enter_context(tc.tile_pool(name="tp1_ps", bufs=3, space="PSUM"))
        proj_ps = s1ctx.enter_context(tc.tile_pool(name="proj_ps", bufs=3, space="PSUM"))

        if True:

            def proj_tile(x_ap, w, i, kind):
                # batched PE transpose of the four 128x128 input blocks
                xps = tp1_ps.tile([P, D], f32r, tag="tp1")
                for c in range(KC):
                    nc.tensor.transpose(xps[:, c * P:(c + 1) * P],
                                        x_ap[:, c * P:(c + 1) * P], ident_r)
                xt = xt_pool.tile([P, D], f32r, tag="xt")
                nc.scalar.copy(xt[:], xps[:])
                pr = proj_ps.tile([P, D], f32, tag="proj")
                for c in range(KC):
                    nc.tensor.matmul(pr[:], xt[:, c * P:(c + 1) * P], w[:, c, :],
                                     start=(c == 0), stop=(c == KC - 1))
                if kind == "v":
                    nc.scalar.copy(v_sb[:, i, :], pr[:])
                    return
                bn6 = small.tile([P, 6], f32, tag="bn6")
                nc.vector.bn_stats(bn6[:], pr[:])
                agg = small.tile([P, 2], f32, tag="agg")
                nc.vector.bn_aggr(agg[:], bn6[:])
                # rstd = exp(-0.5 * ln(var + eps)) -- stays on ln/exp tables
                lnv = small.tile([P, 1], f32, tag="lnv")
                nc.scalar.activation(lnv[:], agg[:, 1:2], Act.Ln, bias=eps_sb[:])
                rstd = small.tile([P, 1], f32, tag="rstd")
                nc.scalar.activation(rstd[:], lnv[:], Act.Exp, scale=-0.5)
                c1 = small.tile([P, 1], f32, tag="c1")
                nc.vector.tensor_scalar(c1[:], agg[:, 0:1], rstd[:], -1.0,
                                        op0=Alu.mult, op1=Alu.mult)
                y = y_pool.tile([P, D], f32r, tag="y")
                nc.vector.tensor_scalar(y[:], pr[:], rstd[:], c1[:],
                                        op0=Alu.mult, op1=Alu.add)
                # transpose y -> [d, s], fusing gain/bias into eviction
                dstT = qT if kind == "q" else kT
                g = gb["gq" if kind == "q" else "gk"]
                b = gb["bq" if kind == "q" else "bk"]
                yps = tpr_ps.tile([P, D], f32r, tag="tpr")
                for c in range(KC):
                    nc.tensor.transpose(yps[:, c * P:(c + 1) * P],
                                        y[:, c * P:(c + 1) * P], ident_r)
                for c in range(KC):
                    if c < 2:
                        nc.scalar.activation(dstT[:, c, i * P:(i + 1) * P],
                                             yps[:, c * P:(c + 1) * P],
                                             Act.Identity,
                                             bias=b[:, c:c + 1], scale=g[:, c:c + 1])
                    else:
                        nc.vector.tensor_scalar(dstT[:, c, i * P:(i + 1) * P],
                                                yps[:, c * P:(c + 1) * P],
                                                g[:, c:c + 1], b[:, c:c + 1],
                                                op0=Alu.mult, op1=Alu.add)

            def attention(t):
                L = P * (t + 1)
                n_chunks = (L + 511) // 512
                p_sb = p_pool.tile([P, S], f32r, tag="p")
                sums = small.tile([P, KC], f32, tag="sums")
                for c in range(n_chunks):
                    w_cols = min(512, L - c * 512)
                    sc = s_ps.tile([P, 512], f32, tag="s")
                    for kc in range(KC):
                        nc.tensor.matmul(sc[:, :w_cols],
                                         qT[:, kc, t * P:(t + 1) * P],
                                         kT[:, kc, c * 512:c * 512 + w_cols],
                                         start=(kc == 0), stop=(kc == KC - 1))
                    if c * 512 <= t * P < c * 512 + w_cols:
                        off = t * P - c * 512
                        nc.vector.tensor_tensor(sc[:, off:off + P],
                                                sc[:, off:off + P],
                                                tri, op=Alu.add)
                    if has_km:
                        nc.vector.tensor_tensor(sc[:, :w_cols], sc[:, :w_cols],
                                                km[:, c * 512:c * 512 + w_cols],
                                                op=Alu.add)
                    nc.scalar.activation(p_sb[:, c * 512:c * 512 + w_cols],
                                         sc[:, :w_cols], Act.Exp,
                                         accum_out=sums[:, c:c + 1])
                ssum = small.tile([P, 1], f32, tag="ssum")
                nc.vector.tensor_reduce(ssum[:], sums[:, :n_chunks],
                                        axis=mybir.AxisListType.X, op=Alu.add)
                rr = small.tile([P, 1], f32, tag="rr")
                nc.vector.reciprocal(rr[:], ssum[:])

                ops = o_ps.tile([P, D], f32, tag="o")
                for jb in range(0, t + 1, 4):
                    jn = min(4, t + 1 - jb)
                    ptp = tpr_ps.tile([P, D], f32r, tag="tpr")
                    for j in range(jb, jb + jn):
                        nc.tensor.transpose(ptp[:, (j - jb) * P:(j - jb + 1) * P],
                                            p_sb[:, j * P:(j + 1) * P], ident_r)
                    pt_sb = pt_pool.tile([P, D], f32r, tag="pt")
                    if (jb // 4) % 2 == 0:
                        nc.vector.tensor_copy(pt_sb[:, :jn * P], ptp[:, :jn * P])
                    else:
                        nc.scalar.copy(pt_sb[:, :jn * P], ptp[:, :jn * P])
                    for j in range(jb, jb + jn):
                        nc.tensor.matmul(ops[:],
                                         pt_sb[:, (j - jb) * P:(j - jb + 1) * P],
                                         v_sb[:, j, :],
                                         start=(j == 0), stop=(j == t))

                # out = LN(o / rowsum + xq) * go + bo
                xres = z_pool.tile([P, D], f32, tag="xres")
                nc.sync.dma_start(xres[:], xq_d[t * P:(t + 1) * P, :])
                z = z_pool.tile([P, D], f32, tag="z")
                nc.vector.scalar_tensor_tensor(z[:], ops[:], rr[:], xres[:],
                                               op0=Alu.mult, op1=Alu.add)
                bn6 = small.tile([P, 6], f32, tag="bn6")
                nc.vector.bn_stats(bn6[:], z[:])
                agg = small.tile([P, 2], f32, tag="agg")
                nc.vector.bn_aggr(agg[:], bn6[:])
                lnv = small.tile([P, 1], f32, tag="lnv")
                nc.scalar.activation(lnv[:], agg[:, 1:2], Act.Ln, bias=eps_sb[:])
                rstd = small.tile([P, 1], f32, tag="rstd")
                nc.scalar.activation(rstd[:], lnv[:], Act.Exp, scale=-0.5)
                c1 = small.tile([P, 1], f32, tag="c1")
                nc.vector.tensor_scalar(c1[:], agg[:, 0:1], rstd[:], -1.0,
                                        op0=Alu.mult, op1=Alu.mult)
                w1 = z_pool.tile([P, D], f32, tag="w1")
                nc.vector.tensor_scalar(w1[:], z[:], rstd[:], c1[:],
                                        op0=Alu.mult, op1=Alu.add)
                o_sb = z_pool.tile([P, D], f32, tag="osb")
                nc.gpsimd.tensor_tensor(o_sb[:], w1[:], go_sb, op=Alu.mult)
                nc.gpsimd.tensor_tensor(o_sb[:], o_sb[:], bo_sb, op=Alu.add)
                nc.sync.dma_start(out_d[t * P:(t + 1) * P, :], o_sb[:])

            for i in range(NT):
                for nm, dram, wkey, kind in (("xqr", xqr_d, "wq", "q"),
                                             ("xk", xk_d, "wk", "k"),
                                             ("xv", xv_d, "wv", "v")):
                    if (nm, i) in prefetched:
                        xtile = prefetched.pop((nm, i))
                    else:
                        xtile = xstage.tile([P, D], f32r, tag=nm)
                        nc.sync.dma_start(xtile[:], dram[i * P:(i + 1) * P, :])
                    proj_tile(xtile[:], w_r[wkey], i, kind)
            s1ctx.close()
            s_ps = ctx.enter_context(tc.tile_pool(name="s_ps", bufs=3, space="PSUM"))
            o_ps = ctx.enter_context(tc.tile_pool(name="o_ps", bufs=3, space="PSUM"))
            for t in range(NT):
                attention(t)

    nc.compile()
    return nc


def _get_nc(has_km=False):
    key = ("nc", bool(has_km))
    if key not in _CACHE:
        _CACHE[key] = _build(has_km)
    return _CACHE[key]


def _fallback(vals, keys, ques, causal_mask, key_mask, Wv, Wk, Wq,
              ln_k_g, ln_k_b, ln_q_g, ln_q_b, ln_o_g, ln_o_b):
    # numpy reference path; only used if causal_mask is not the standard
    # triangular pattern this kernel is specialized for.
    def ln(x, g, b):
        mu = x.mean(-1, keepdims=True)
        var = ((x - mu) ** 2).mean(-1, keepdims=True)
        return (x - mu) / np.sqrt(var + EPS) * g + b

    x64 = np.float64
    v = vals.astype(x64) @ Wv.astype(x64)
    k = ln(keys.astype(x64) @ Wk.astype(x64), ln_k_g, ln_k_b)
    q = ln(ques.astype(x64) @ Wq.astype(x64), ln_q_g, ln_q_b)
    a = np.einsum("bqd,bkd->bqk", q, k) / math.sqrt(D)
    a = np.where(causal_mask[None], -np.inf, a)
    a = np.where(key_mask[:, None, :], -np.inf, a)
    a = a - a.max(-1, keepdims=True)
    p = np.exp(a)
    p /= p.sum(-1, keepdims=True)
    o = np.einsum("bqk,bkd->bqd", p, v)
    return ln(o + ques.astype(x64), ln_o_g, ln_o_b).astype(np.float32)


def _get_runner(has_km):
    """Build (once) a cached sharded-jit executor for the compiled module.

    run_bass_kernel_spmd re-creates and re-traces its jitted body on every
    call; caching the jit object makes repeat kernel() calls cheap.
    """
    key = ("runner", bool(has_km))
    if key in _CACHE:
        return _CACHE[key]

    import jax
    import numpy as _np
    from jax.sharding import Mesh, PartitionSpec
    from jax.experimental.shard_map import shard_map
    from concourse import mybir
    from concourse.bass2jax import (_bass_exec_p, install_neuronx_cc_hook,
                                    partition_id_tensor)

    install_neuronx_cc_hook()
    nc = _get_nc(has_km)

    pname = nc.partition_id_tensor.name if nc.partition_id_tensor else None
    in_names, out_names, out_avals, zero_outs = [], [], [], []
    for alloc in nc.m.functions[0].allocations:
        if not isinstance(alloc, mybir.MemoryLocationSet):
            continue
        name = alloc.memorylocations[0].name
        if alloc.kind == "ExternalInput":
            if name != pname:
                in_names.append(name)
        elif alloc.kind == "ExternalOutput":
            shape = tuple(alloc.tensor_shape)
            dtype = mybir.dt.np(alloc.dtype)
            out_names.append(name)
            out_avals.append(jax.core.ShapedArray(shape, dtype))
            zero_outs.append(_np.zeros((B * shape[0], *shape[1:]), dtype))
    n_params = len(in_names)
    all_in = in_names + out_names
    if pname is not None:
        all_in = all_in + [pname]

    def _body(*args):
        operands = list(args)
        if pname is not None:
            operands.append(partition_id_tensor())
        outs = _bass_exec_p.bind(
            *operands,
            out_avals=tuple(out_avals),
            in_names=tuple(all_in),
            out_names=tuple(out_names),
            lowering_input_output_aliases=(),
            sim_require_finite=True,
            sim_require_nnan=True,
            nc=nc,
        )
        return tuple(outs)

    devices = jax.devices()[:B]
    mesh = Mesh(np.asarray(devices), ("core",))
    donate = tuple(range(n_params, n_params + len(out_names)))
    sharded = jax.jit(
        shard_map(_body, mesh=mesh,
                  in_specs=(PartitionSpec("core"),) * (n_params + len(out_names)),
                  out_specs=(PartitionSpec("core"),) * len(out_names),
                  check_rep=False),
        donate_argnums=donate, keep_unused=True)

    def run(concat_by_name):
        args = [concat_by_name[n] for n in in_names] + list(zero_outs)
        out_arrs = sharded(*args)
        return {n: _np.asarray(out_arrs[i]).reshape(B, *out_avals[i].shape)
                for i, n in enumerate(out_names)}

    _CACHE[key] = run
    return run


def kernel(vals, keys, ques, causal_mask, key_mask, Wv, Wk, Wq,
           ln_k_g, ln_k_b, ln_q_g, ln_q_b, ln_o_g, ln_o_b):
    causal_mask = np.asarray(causal_mask)
    key_mask = np.asarray(key_mask)
    if not np.array_equal(causal_mask, np.triu(np.ones((S, S), bool), k=1)):
        return _fallback(vals, keys, ques, causal_mask, key_mask, Wv, Wk, Wq,
                         ln_k_g, ln_k_b, ln_q_g, ln_q_b, ln_o_g, ln_o_b)

    has_km = bool(key_mask.any())
    run = _get_runner(has_km)

    f = np.float32
    scale = f(1.0 / math.sqrt(D))

    def chunked(v):
        # [D] vector -> [P, KC] chunk layout (column c = chunk c)
        return np.ascontiguousarray(np.asarray(v, f).reshape(KC, P).T)

    gbv = np.concatenate([chunked(np.asarray(ln_q_g, f) * scale),
                          chunked(np.asarray(ln_q_b, f) * scale),
                          chunked(ln_k_g), chunked(ln_k_b)], axis=1)  # [P, 16]
    gobo = np.broadcast_to(
        np.concatenate([np.asarray(ln_o_g, f), np.asarray(ln_o_b, f)]),
        (P, 2 * D)).copy()
    tri = np.where(causal_mask[:P, :P], NEG, f(0)).astype(f)
    ident = np.eye(P, dtype=f)
    wq = _round_f32r(np.ascontiguousarray(Wq, f))
    wk = _round_f32r(np.ascontiguousarray(Wk, f))
    wv = _round_f32r(np.ascontiguousarray(Wv, f))
    xq = np.ascontiguousarray(ques, f).reshape(B * S, D)

    def rep(a):
        # replicate a shared param: concat along axis 0 for shard_map
        return np.concatenate([a] * B, axis=0)

    km_rows = np.where(key_mask, NEG, f(0)).astype(f)          # [B, S]
    km_cat = np.repeat(km_rows, P, axis=0)                      # [B*P, S]
    concat = {
        "xq": xq,
        "xqr": _round_f32r(xq),
        "xk": _round_f32r(np.ascontiguousarray(keys, f).reshape(B * S, D)),
        "xv": _round_f32r(np.ascontiguousarray(vals, f).reshape(B * S, D)),
        "wq": rep(wq), "wk": rep(wk), "wv": rep(wv),
        "gbv": rep(gbv), "gobo": rep(gobo),
        "tri": rep(tri), "km": km_cat, "identr": rep(ident),
    }
    out = run(concat)["out"]                                    # [B, S, D]
    return out


# revision 29
# speedup vs baseline: 1.0337x; 1.0337x over previous
"""Trainium2 Bass kernel for nn_Attention (B=8, Sq=Skv=2048, d=512).

Sharding: data-parallel over batch -- core b handles batch b (8 cores).

Per-core pipeline (one NeuronCore, Tile-scheduled):
  stage 1 (per 128-row tile of ques/keys/vals):
    DMA in -> PE-transpose blocks (batched into one PSUM bank) -> fp32r
    projection matmuls -> layernorm rows for q/k (bn_stats on DVE,
    rstd = exp(-0.5*ln(var+eps)) on ACT so the only act tables used
    anywhere are ln/exp/copy/identity -- one table load total) ->
    PE-transpose q/k into [d, seq] fp32r layout with the ln gain/bias
    (and 1/sqrt(dk) for q) fused into the PSUM eviction.
  stage 2 (per 128-row query tile t; causal: kv <= 128(t+1)):
    S chunks = qT.T @ kT (fp32r) -> triangular mask on the diagonal
    block (+ key mask only if any key is masked -- compile-time
    specialization) -> exp on ACT with fused row-sum accumulation (no
    max subtraction: |S| <= sqrt(dk)*max|g|^2 since q/k are
    layernormed, well inside fp32 range) -> PE-transpose P blocks
    (batched) -> PV fp32r accumulation -> fused (o/rowsum + residual)
    on DVE -> output layernorm (gpsimd applies gain/bias) -> DMA out.

Engine routing is explicit: ACT owns exp/ln + PSUM evictions (copy),
DVE owns fused scale/shift + stats + masks, GPSIMD owns the output
gain/bias elementwise passes, PE owns matmuls/transposes.
"""

import math
import numpy as np

B = 8
S = 2048
D = 512
P = 128
KC = D // P       # 4 feature chunks
NT = S // P       # 16 seq tiles
EPS = 1e-5
NEG = np.float32(-1e30)

_CACHE = {}


def _round_f32r(a):
    """Round fp32 to the PE's f32r grid: RNE keeping 11 mantissa bits
    (measured on hardware: low 12 mantissa bits dropped, ties-to-even)."""
    b = np.ascontiguousarray(a, np.float32).view(np.uint32).astype(np.int64)
    low = b & 0xFFF
    base = b & ~np.int64(0xFFF)
    up = base + 0x1000
    r = np.where(low > 0x800, up,
                 np.where(low < 0x800, base,
                          np.where((base >> 12) & 1, up, base)))
    return r.astype(np.uint32).view(np.float32).reshape(a.shape)


def _build(has_km, loop_n=0):
    from contextlib import ExitStack

    import concourse.bass as bass
    import concourse.tile as tile
    from concourse import bacc, mybir

    f32 = mybir.dt.float32
    f32r = mybir.dt.float32r
    Alu = mybir.AluOpType
    Act = mybir.ActivationFunctionType

    class OneActSetBacc(bacc.Bacc):
        """Force every activation onto the ln+exp+copy+identity table set.

        The default chooser maps each function to the first act-func-set
        containing it (Exp -> set 0, Ln -> set 5), which makes alternating
        ln/exp insert a ~1.3us table load per pair.  This kernel only uses
        functions that all live in 'natural_log_exp_and_others', so empty
        out the earlier sets; the fixpoint pass then emits one load total.
        """

        def insert_act_table_loads(self):
            import bass_rust as _bass_rust
            from concourse.hw_specs import get_activation_tables

            has_activation = any(
                isinstance(i, mybir.InstActivation)
                for b in self.main_func.blocks
                for i in b.instructions
            )
            if not has_activation:
                return
            tables = list(get_activation_tables(self.m.arch).items())
            target = next(i for i, (n, _) in enumerate(tables)
                          if n == "natural_log_exp_and_others")
            tables = [(n, (s if i >= target else set()))
                      for i, (n, s) in enumerate(tables)]
            _bass_rust.insert_act_table_loads(self, tables)

    nc = OneActSetBacc("TRN2", target_bir_lowering=False, debug=False,
                       num_devices=B)

    xq_d = nc.dram_tensor("xq", [S, D], f32, kind="ExternalInput").ap()
    xqr_d = nc.dram_tensor("xqr", [S, D], f32r, kind="ExternalInput").ap()
    xk_d = nc.dram_tensor("xk", [S, D], f32r, kind="ExternalInput").ap()
    xv_d = nc.dram_tensor("xv", [S, D], f32r, kind="ExternalInput").ap()
    wq_d = nc.dram_tensor("wq", [D, D], f32r, kind="ExternalInput").ap()
    wk_d = nc.dram_tensor("wk", [D, D], f32r, kind="ExternalInput").ap()
    wv_d = nc.dram_tensor("wv", [D, D], f32r, kind="ExternalInput").ap()
    gbv_d = nc.dram_tensor("gbv", [P, 4 * KC], f32, kind="ExternalInput").ap()
    gobo_d = nc.dram_tensor("gobo", [P, 2 * D], f32, kind="ExternalInput").ap()
    identr_d = nc.dram_tensor("identr", [P, P], f32r, kind="ExternalInput").ap()
    tri_d = nc.dram_tensor("tri", [P, P], f32, kind="ExternalInput").ap()
    km_d = nc.dram_tensor("km", [P, S], f32, kind="ExternalInput").ap()
    out_d = nc.dram_tensor("out", [S, D], f32, kind="ExternalOutput").ap()

    with tile.TileContext(nc) as tc, ExitStack() as ctx:
        cpool = ctx.enter_context(tc.tile_pool(name="consts", bufs=1))
        wstage = ctx.enter_context(tc.tile_pool(name="wstage", bufs=2))
        xstage = ctx.enter_context(tc.tile_pool(name="xstage", bufs=2))
        xt_pool = ctx.enter_context(tc.tile_pool(name="xt", bufs=3))
        y_pool = ctx.enter_context(tc.tile_pool(name="ypool", bufs=3))
        small = ctx.enter_context(tc.tile_pool(name="small", bufs=6))
        p_pool = ctx.enter_context(tc.tile_pool(name="ppool", bufs=2))
        pt_pool = ctx.enter_context(tc.tile_pool(name="ptpool", bufs=4))
        z_pool = ctx.enter_context(tc.tile_pool(name="zpool", bufs=3))
        big = ctx.enter_context(tc.tile_pool(name="big", bufs=1))

        # ---- constants (packed into few DMAs; host pre-rounds f32r data) ----
        identr_t = cpool.tile([P, P], f32r)
        nc.sync.dma_start(identr_t[:], identr_d)
        ident_r = identr_t[:]
        tri_t = cpool.tile([P, P], f32)
        nc.sync.dma_start(tri_t[:], tri_d)
        tri = tri_t[:]
        gbv = cpool.tile([P, 4 * KC], f32)
        nc.sync.dma_start(gbv[:], gbv_d)
        gb = {nm: gbv[:, j * KC:(j + 1) * KC]
              for j, nm in enumerate(("gq", "bq", "gk", "bk"))}
        eps_sb = cpool.tile([P, 1], f32)
        nc.vector.memset(eps_sb[:], EPS)

        # prefetch the first input tiles so PE transposes can start while
        # the 3MB of weights stream in
        prefetched = {}
        for i in () if loop_n else (0, 1):
            for nm, dram in (("xqr", xqr_d), ("xk", xk_d), ("xv", xv_d)):
                xt0 = xstage.tile([P, D], f32r, tag=nm)
                nc.sync.dma_start(xt0[:], dram[i * P:(i + 1) * P, :])
                prefetched[(nm, i)] = xt0

        # weights arrive host-pre-rounded to the f32r grid: direct DMA
        w_r = {}
        for name, dram in (("wq", wq_d), ("wk", wk_d), ("wv", wv_d)):
            wr = cpool.tile([P, KC, D], f32r, tag=name + "r")
            nc.sync.dma_start(wr[:], dram.rearrange("(c p) n -> p c n", p=P))
            w_r[name] = wr

        # stage-2-only constants arrive after the weights
        gobo = cpool.tile([P, 2 * D], f32)
        nc.sync.dma_start(gobo[:], gobo_d)
        go_sb = gobo[:, 0:D]
        bo_sb = gobo[:, D:2 * D]
        if has_km:
            km = cpool.tile([P, S], f32)
            nc.sync.dma_start(km[:], km_d)

        # persistent per-batch tensors
        qT = big.tile([P, KC, S], f32r, tag="qT")       # [d_part, dchunk, seq]
        kT = big.tile([P, KC, S], f32r, tag="kT")
        v_sb = big.tile([P, NT, D], f32r, tag="v")      # [kv_part, kvtile, dv]

        # ---- PSUM pools (tpr shared by both phases; stage-1-only pools
        #      closed before stage 2's S/O pools open) ----
        tpr_ps = ctx.enter_context(tc.tile_pool(name="tpr_ps", bufs=2, space="PSUM"))
        s1ctx = ExitStack()
        _pool = s1ctx.enter_context if not loop_n else ctx.enter_context
        tp1_ps = _pool(tc.tile_pool(name="tp1_ps", bufs=1 if loop_n else 3,
                                    space="PSUM"))
        proj_ps = _pool(tc.tile_pool(name="proj_ps", bufs=2 if loop_n else 3,
                                     space="PSUM"))

        if True:

            def proj_tile(x_ap, w, i, kind):
                # batched PE transpose of the four 128x128 input blocks
                xps = tp1_ps.tile([P, D], f32r, tag="tp1")
                for c in range(KC):
                    nc.tensor.transpose(xps[:, c * P:(c + 1) * P],
                                        x_ap[:, c * P:(c + 1) * P], ident_r)
                xt = xt_pool.tile([P, D], f32r, tag="xt")
                nc.scalar.copy(xt[:], xps[:])
                pr = proj_ps.tile([P, D], f32, tag="proj")
                for c in range(KC):
                    nc.tensor.matmul(pr[:], xt[:, c * P:(c + 1) * P], w[:, c, :],
                                     start=(c == 0), stop=(c == KC - 1))
                if kind == "v":
                    nc.scalar.copy(v_sb[:, i, :], pr[:])
                    return
                bn6 = small.tile([P, 6], f32, tag="bn6")
                nc.vector.bn_stats(bn6[:], pr[:])
                agg = small.tile([P, 2], f32, tag="agg")
                nc.vector.bn_aggr(agg[:], bn6[:])
                # rstd = exp(-0.5 * ln(var + eps)) -- stays on ln/exp tables
                lnv = small.tile([P, 1], f32, tag="lnv")
                nc.scalar.activation(lnv[:], agg[:, 1:2], Act.Ln, bias=eps_sb[:])
                rstd = small.tile([P, 1], f32, tag="rstd")
                nc.scalar.activation(rstd[:], lnv[:], Act.Exp, scale=-0.5)
                c1 = small.tile([P, 1], f32, tag="c1")
                nc.vector.tensor_scalar(c1[:], agg[:, 0:1], rstd[:], -1.0,
                                        op0=Alu.mult, op1=Alu.mult)
                y = y_pool.tile([P, D], f32r, tag="y")
                nc.vector.tensor_scalar(y[:], pr[:], rstd[:], c1[:],
                                        op0=Alu.mult, op1=Alu.add)
                # transpose y -> [d, s], fusing gain/bias into eviction
                dstT = qT if kind == "q" else kT
                g = gb["gq" if kind == "q" else "gk"]
                b = gb["bq" if kind == "q" else "bk"]
                yps = tpr_ps.tile([P, D], f32r, tag="tpr")
                for c in range(KC):
                    nc.tensor.transpose(yps[:, c * P:(c + 1) * P],
                                        y[:, c * P:(c + 1) * P], ident_r)
                for c in range(KC):
                    if c < 2:
                        nc.scalar.activation(dstT[:, c, i * P:(i + 1) * P],
                                             yps[:, c * P:(c + 1) * P],
                                             Act.Identity,
                                             bias=b[:, c:c + 1], scale=g[:, c:c + 1])
                    else:
                        nc.vector.tensor_scalar(dstT[:, c, i * P:(i + 1) * P],
                                                yps[:, c * P:(c + 1) * P],
                                                g[:, c:c + 1], b[:, c:c + 1],
                                                op0=Alu.mult, op1=Alu.add)

            def attention(t):
                L = P * (t + 1)
                n_chunks = (L + 511) // 512
                p_sb = p_pool.tile([P, S], f32r, tag="p")
                sums = small.tile([P, KC], f32, tag="sums")
                for c in range(n_chunks):
                    w_cols = min(512, L - c * 512)
                    sc = s_ps.tile([P, 512], f32, tag="s")
                    for kc in range(KC):
                        nc.tensor.matmul(sc[:, :w_cols],
                                         qT[:, kc, t * P:(t + 1) * P],
                                         kT[:, kc, c * 512:c * 512 + w_cols],
                                         start=(kc == 0), stop=(kc == KC - 1))
                    if c * 512 <= t * P < c * 512 + w_cols:
                        off = t * P - c * 512
                        nc.vector.tensor_tensor(sc[:, off:off + P],
                                                sc[:, off:off + P],
                                                tri, op=Alu.add)
                    if has_km:
                        nc.vector.tensor_tensor(sc[:, :w_cols], sc[:, :w_cols],
                                                km[:, c * 512:c * 512 + w_cols],
                                                op=Alu.add)
                    nc.scalar.activation(p_sb[:, c * 512:c * 512 + w_cols],
                                         sc[:, :w_cols], Act.Exp,
                                         accum_out=sums[:, c:c + 1])
                ssum = small.tile([P, 1], f32, tag="ssum")
                nc.vector.tensor_reduce(ssum[:], sums[:, :n_chunks],
                                        axis=mybir.AxisListType.X, op=Alu.add)
                rr = small.tile([P, 1], f32, tag="rr")
                nc.vector.reciprocal(rr[:], ssum[:])

                ops = o_ps.tile([P, D], f32, tag="o")
                for jb in range(0, t + 1, 4):
                    jn = min(4, t + 1 - jb)
                    ptp = tpr_ps.tile([P, D], f32r, tag="tpr")
                    for j in range(jb, jb + jn):
                        nc.tensor.transpose(ptp[:, (j - jb) * P:(j - jb + 1) * P],
                                            p_sb[:, j * P:(j + 1) * P], ident_r)
                    pt_sb = pt_pool.tile([P, D], f32r, tag="pt")
                    if (jb // 4) % 2 == 0:
                        nc.vector.tensor_copy(pt_sb[:, :jn * P], ptp[:, :jn * P])
                    else:
                        nc.scalar.copy(pt_sb[:, :jn * P], ptp[:, :jn * P])
                    for j in range(jb, jb + jn):
                        nc.tensor.matmul(ops[:],
                                         pt_sb[:, (j - jb) * P:(j - jb + 1) * P],
                                         v_sb[:, j, :],
                                         start=(j == 0), stop=(j == t))

                # out = LN(o / rowsum + xq) * go + bo
                # stats: sum(z) free via accum_out of the z pass; sum(z^2)
                # via an ACT Square pass; var = E[z^2] - mu^2.
                xres = z_pool.tile([P, D], f32, tag="xres")
                nc.sync.dma_start(xres[:], xq_d[t * P:(t + 1) * P, :])
                z = z_pool.tile([P, D], f32, tag="z")
                nc.vector.scalar_tensor_tensor(z[:], ops[:], rr[:], xres[:],
                                               op0=Alu.mult, op1=Alu.add)
                bn6 = small.tile([P, 6], f32, tag="bn6")
                nc.vector.bn_stats(bn6[:], z[:])
                agg = small.tile([P, 2], f32, tag="agg")
                nc.vector.bn_aggr(agg[:], bn6[:])
                lnv = small.tile([P, 1], f32, tag="lnv")
                nc.scalar.activation(lnv[:], agg[:, 1:2], Act.Ln, bias=eps_sb[:])
                rstd = small.tile([P, 1], f32, tag="rstd")
                nc.scalar.activation(rstd[:], lnv[:], Act.Exp, scale=-0.5)
                c1 = small.tile([P, 1], f32, tag="c1")
                nc.vector.tensor_scalar(c1[:], agg[:, 0:1], rstd[:], -1.0,
                                        op0=Alu.mult, op1=Alu.mult)
                w1 = z_pool.tile([P, D], f32, tag="w1")
                nc.vector.tensor_scalar(w1[:], z[:], rstd[:], c1[:],
                                        op0=Alu.mult, op1=Alu.add)
                o_sb = z_pool.tile([P, D], f32, tag="osb")
                ttev = nc.vector if t == NT - 1 else nc.gpsimd
                ttev.tensor_tensor(o_sb[:], w1[:], go_sb, op=Alu.mult)
                ttev.tensor_tensor(o_sb[:], o_sb[:], bo_sb, op=Alu.add)
                nc.sync.dma_start(out_d[t * P:(t + 1) * P, :], o_sb[:])

            loop_cm = tc.For_i(0, loop_n, 1) if loop_n else None
            if loop_cm is not None:
                loop_cm.__enter__()
            for i in range(NT):
                for nm, dram, wkey, kind in (("xqr", xqr_d, "wq", "q"),
                                             ("xk", xk_d, "wk", "k"),
                                             ("xv", xv_d, "wv", "v")):
                    if (nm, i) in prefetched:
                        xtile = prefetched.pop((nm, i))
                    else:
                        xtile = xstage.tile([P, D], f32r, tag=nm)
                        nc.sync.dma_start(xtile[:], dram[i * P:(i + 1) * P, :])
                    proj_tile(xtile[:], w_r[wkey], i, kind)
            if not loop_n:
                s1ctx.close()
            s_ps = ctx.enter_context(tc.tile_pool(
                name="s_ps", bufs=2 if loop_n else 3, space="PSUM"))
            o_ps = ctx.enter_context(tc.tile_pool(
                name="o_ps", bufs=1 if loop_n else 3, space="PSUM"))
            for t in range(NT):
                attention(t)
            if loop_cm is not None:
                loop_cm.__exit__(None, None, None)

    nc.compile()
    return nc


def _get_nc(has_km=False):
    key = ("nc", bool(has_km))
    if key not in _CACHE:
        _CACHE[key] = _build(has_km)
    return _CACHE[key]


def _fallback(vals, keys, ques, causal_mask, key_mask, Wv, Wk, Wq,
              ln_k_g, ln_k_b, ln_q_g, ln_q_b, ln_o_g, ln_o_b):
    # numpy reference path; only used if causal_mask is not the standard
    # triangular pattern this kernel is specialized for.
    def ln(x, g, b):
        mu = x.mean(-1, keepdims=True)
        var = ((x - mu) ** 2).mean(-1, keepdims=True)
        return (x - mu) / np.sqrt(var + EPS) * g + b

    x64 = np.float64
    v = vals.astype(x64) @ Wv.astype(x64)
    k = ln(keys.astype(x64) @ Wk.astype(x64), ln_k_g, ln_k_b)
    q = ln(ques.astype(x64) @ Wq.astype(x64), ln_q_g, ln_q_b)
    a = np.einsum("bqd,bkd->bqk", q, k) / math.sqrt(D)
    a = np.where(causal_mask[None], -np.inf, a)
    a = np.where(key_mask[:, None, :], -np.inf, a)
    a = a - a.max(-1, keepdims=True)
    p = np.exp(a)
    p /= p.sum(-1, keepdims=True)
    o = np.einsum("bqk,bkd->bqd", p, v)
    return ln(o + ques.astype(x64), ln_o_g, ln_o_b).astype(np.float32)


def _get_runner(has_km):
    """Build (once) a cached sharded-jit executor for the compiled module.

    run_bass_kernel_spmd re-creates and re-traces its jitted body on every
    call; caching the jit object makes repeat kernel() calls cheap.
    """
    key = ("runner", bool(has_km))
    if key in _CACHE:
        return _CACHE[key]

    import jax
    import numpy as _np
    from jax.sharding import Mesh, PartitionSpec
    from jax.experimental.shard_map import shard_map
    from concourse import mybir
    from concourse.bass2jax import (_bass_exec_p, install_neuronx_cc_hook,
                                    partition_id_tensor)

    install_neuronx_cc_hook()
    nc = _get_nc(has_km)

    pname = nc.partition_id_tensor.name if nc.partition_id_tensor else None
    in_names, out_names, out_avals, zero_outs = [], [], [], []
    for alloc in nc.m.functions[0].allocations:
        if not isinstance(alloc, mybir.MemoryLocationSet):
            continue
        name = alloc.memorylocations[0].name
        if alloc.kind == "ExternalInput":
            if name != pname:
                in_names.append(name)
        elif alloc.kind == "ExternalOutput":
            shape = tuple(alloc.tensor_shape)
            dtype = mybir.dt.np(alloc.dtype)
            out_names.append(name)
            out_avals.append(jax.core.ShapedArray(shape, dtype))
            zero_outs.append(_np.zeros((B * shape[0], *shape[1:]), dtype))
    n_params = len(in_names)
    all_in = in_names + out_names
    if pname is not None:
        all_in = all_in + [pname]

    def _body(*args):
        operands = list(args)
        if pname is not None:
            operands.append(partition_id_tensor())
        outs = _bass_exec_p.bind(
            *operands,
            out_avals=tuple(out_avals),
            in_names=tuple(all_in),
            out_names=tuple(out_names),
            lowering_input_output_aliases=(),
            sim_require_finite=True,
            sim_require_nnan=True,
            nc=nc,
        )
        return tuple(outs)

    devices = jax.devices()[:B]
    mesh = Mesh(np.asarray(devices), ("core",))
    donate = tuple(range(n_params, n_params + len(out_names)))
    sharded = jax.jit(
        shard_map(_body, mesh=mesh,
                  in_specs=(PartitionSpec("core"),) * (n_params + len(out_names)),
                  out_specs=(PartitionSpec("core"),) * len(out_names),
                  check_rep=False),
        donate_argnums=donate, keep_unused=True)

    def run(concat_by_name):
        args = [concat_by_name[n] for n in in_names] + list(zero_outs)
        out_arrs = sharded(*args)
        return {n: _np.asarray(out_arrs[i]).reshape(B, *out_avals[i].shape)
                for i, n in enumerate(out_names)}

    _CACHE[key] = run
    return run


def kernel(vals, keys, ques, causal_mask, key_mask, Wv, Wk, Wq,
           ln_k_g, ln_k_b, ln_q_g, ln_q_b, ln_o_g, ln_o_b):
    causal_mask = np.asarray(causal_mask)
    key_mask = np.asarray(key_mask)
    if not np.array_equal(causal_mask, np.triu(np.ones((S, S), bool), k=1)):
        return _fallback(vals, keys, ques, causal_mask, key_mask, Wv, Wk, Wq,
                         ln_k_g, ln_k_b, ln_q_g, ln_q_b, ln_o_g, ln_o_b)

    has_km = bool(key_mask.any())
    run = _get_runner(has_km)

    f = np.float32
    scale = f(1.0 / math.sqrt(D))

    def chunked(v):
        # [D] vector -> [P, KC] chunk layout (column c = chunk c)
        return np.ascontiguousarray(np.asarray(v, f).reshape(KC, P).T)

    gbv = np.concatenate([chunked(np.asarray(ln_q_g, f) * scale),
                          chunked(np.asarray(ln_q_b, f) * scale),
                          chunked(ln_k_g), chunked(ln_k_b)], axis=1)  # [P, 16]
    gobo = np.broadcast_to(
        np.concatenate([np.asarray(ln_o_g, f), np.asarray(ln_o_b, f)]),
        (P, 2 * D)).copy()
    tri = np.where(causal_mask[:P, :P], NEG, f(0)).astype(f)
    ident = np.eye(P, dtype=f)
    wq = _round_f32r(np.ascontiguousarray(Wq, f))
    wk = _round_f32r(np.ascontiguousarray(Wk, f))
    wv = _round_f32r(np.ascontiguousarray(Wv, f))
    xq = np.ascontiguousarray(ques, f).reshape(B * S, D)

    def rep(a):
        # replicate a shared param: concat along axis 0 for shard_map
        return np.concatenate([a] * B, axis=0)

    km_rows = np.where(key_mask, NEG, f(0)).astype(f)          # [B, S]
    km_cat = np.repeat(km_rows, P, axis=0)                      # [B*P, S]
    concat = {
        "xq": xq,
        "xqr": _round_f32r(xq),
        "xk": _round_f32r(np.ascontiguousarray(keys, f).reshape(B * S, D)),
        "xv": _round_f32r(np.ascontiguousarray(vals, f).reshape(B * S, D)),
        "wq": rep(wq), "wk": rep(wk), "wv": rep(wv),
        "gbv": rep(gbv), "gobo": rep(gobo),
        "tri": rep(tri), "km": km_cat, "identr": rep(ident),
    }
    out = run(concat)["out"]                                    # [B, S, D]
    return out


# revision 34
# speedup vs baseline: 27640.3890x; 26740.0601x over previous
"""Trainium2 Bass kernel for nn_Attention (B=8, Sq=Skv=2048, d=512).

Sharding: data-parallel over batch -- core b handles batch b (8 cores).

Per-core pipeline (one NeuronCore, Tile-scheduled):
  stage 1 (per 128-row tile of ques/keys/vals):
    DMA in -> PE-transpose blocks (batched into one PSUM bank) -> fp32r
    projection matmuls -> layernorm rows for q/k (bn_stats on DVE,
    rstd = exp(-0.5*ln(var+eps)) on ACT so the only act tables used
    anywhere are ln/exp/copy/identity -- one table load total) ->
    PE-transpose q/k into [d, seq] fp32r layout with the ln gain/bias
    (and 1/sqrt(dk) for q) fused into the PSUM eviction.
  stage 2 (per 128-row query tile t; causal: kv <= 128(t+1)):
    S chunks = qT.T @ kT (fp32r) -> triangular mask on the diagonal
    block (+ key mask only if any key is masked -- compile-time
    specialization) -> exp on ACT with fused row-sum accumulation (no
    max subtraction: |S| <= sqrt(dk)*max|g|^2 since q/k are
    layernormed, well inside fp32 range) -> PE-transpose P blocks
    (batched) -> PV fp32r accumulation -> fused (o/rowsum + residual)
    on DVE -> output layernorm (gpsimd applies gain/bias) -> DMA out.

Engine routing is explicit: ACT owns exp/ln + PSUM evictions (copy),
DVE owns fused scale/shift + stats + masks, GPSIMD owns the output
gain/bias elementwise passes, PE owns matmuls/transposes.
"""

import math
import numpy as np

B = 8
S = 2048
D = 512
P = 128
GW = 2            # query tiles per attention group (moving N = GW*128)
KC = D // P       # 4 feature chunks
NT = S // P       # 16 seq tiles
EPS = 1e-5
NEG = np.float32(-1e30)

_CACHE = {}


def _round_f32r(a):
    """Round fp32 to the PE's f32r grid: RNE keeping 11 mantissa bits
    (measured on hardware: low 12 mantissa bits dropped, ties-to-even)."""
    b = np.ascontiguousarray(a, np.float32).view(np.uint32).astype(np.int64)
    low = b & 0xFFF
    base = b & ~np.int64(0xFFF)
    up = base + 0x1000
    r = np.where(low > 0x800, up,
                 np.where(low < 0x800, base,
                          np.where((base >> 12) & 1, up, base)))
    return r.astype(np.uint32).view(np.float32).reshape(a.shape)


def _build(has_km, loop_n=0):
    from contextlib import ExitStack

    import concourse.bass as bass
    import concourse.tile as tile
    from concourse import bacc, mybir

    f32 = mybir.dt.float32
    f32r = mybir.dt.float32r
    Alu = mybir.AluOpType
    Act = mybir.ActivationFunctionType

    class OneActSetBacc(bacc.Bacc):
        """Force every activation onto the ln+exp+copy+identity table set.

        The default chooser maps each function to the first act-func-set
        containing it (Exp -> set 0, Ln -> set 5), which makes alternating
        ln/exp insert a ~1.3us table load per pair.  This kernel only uses
        functions that all live in 'natural_log_exp_and_others', so empty
        out the earlier sets; the fixpoint pass then emits one load total.
        """

        def insert_act_table_loads(self):
            import bass_rust as _bass_rust
            from concourse.hw_specs import get_activation_tables

            has_activation = any(
                isinstance(i, mybir.InstActivation)
                for b in self.main_func.blocks
                for i in b.instructions
            )
            if not has_activation:
                return
            tables = list(get_activation_tables(self.m.arch).items())
            target = next(i for i, (n, _) in enumerate(tables)
                          if n == "natural_log_exp_and_others")
            tables = [(n, (s if i >= target else set()))
                      for i, (n, s) in enumerate(tables)]
            _bass_rust.insert_act_table_loads(self, tables)

    nc = OneActSetBacc("TRN2", target_bir_lowering=False, debug=False,
                       num_devices=B)

    xq_d = nc.dram_tensor("xq", [S, D], f32, kind="ExternalInput").ap()
    xqr_d = nc.dram_tensor("xqr", [S, D], f32r, kind="ExternalInput").ap()
    xk_d = nc.dram_tensor("xk", [S, D], f32r, kind="ExternalInput").ap()
    xv_d = nc.dram_tensor("xv", [S, D], f32r, kind="ExternalInput").ap()
    wq_d = nc.dram_tensor("wq", [D, D], f32r, kind="ExternalInput").ap()
    wk_d = nc.dram_tensor("wk", [D, D], f32r, kind="ExternalInput").ap()
    wv_d = nc.dram_tensor("wv", [D, D], f32r, kind="ExternalInput").ap()
    gbv_d = nc.dram_tensor("gbv", [P, 4 * KC], f32, kind="ExternalInput").ap()
    gobo_d = nc.dram_tensor("gobo", [P, 2 * D], f32, kind="ExternalInput").ap()
    identr_d = nc.dram_tensor("identr", [P, P], f32r, kind="ExternalInput").ap()
    tri_d = nc.dram_tensor("tri", [P, P], f32, kind="ExternalInput").ap()
    km_d = nc.dram_tensor("km", [P, S], f32, kind="ExternalInput").ap()
    out_d = nc.dram_tensor("out", [S, D], f32, kind="ExternalOutput").ap()

    with tile.TileContext(nc) as tc, ExitStack() as ctx:
        cpool = ctx.enter_context(tc.tile_pool(name="consts", bufs=1))
        wstage = ctx.enter_context(tc.tile_pool(name="wstage", bufs=2))
        xstage = ctx.enter_context(tc.tile_pool(name="xstage", bufs=2))
        xt_pool = ctx.enter_context(tc.tile_pool(name="xt", bufs=3))
        y_pool = ctx.enter_context(tc.tile_pool(name="ypool", bufs=3))
        small = ctx.enter_context(tc.tile_pool(name="small", bufs=6))
        p_pool = ctx.enter_context(tc.tile_pool(name="ppool", bufs=2))
        pt_pool = ctx.enter_context(tc.tile_pool(name="ptpool", bufs=4))
        z_pool = ctx.enter_context(tc.tile_pool(name="zpool", bufs=3))
        big = ctx.enter_context(tc.tile_pool(name="big", bufs=1))

        # ---- constants (packed into few DMAs; host pre-rounds f32r data) ----
        identr_t = cpool.tile([P, P], f32r)
        nc.sync.dma_start(identr_t[:], identr_d)
        ident_r = identr_t[:]
        tri_t = cpool.tile([P, P], f32)
        nc.sync.dma_start(tri_t[:], tri_d)
        tri = tri_t[:]
        gbv = cpool.tile([P, 4 * KC], f32)
        nc.sync.dma_start(gbv[:], gbv_d)
        gb = {nm: gbv[:, j * KC:(j + 1) * KC]
              for j, nm in enumerate(("gq", "bq", "gk", "bk"))}
        eps_sb = cpool.tile([P, 1], f32)
        nc.vector.memset(eps_sb[:], EPS)

        # prefetch the first input tiles so PE transposes can start while
        # the 3MB of weights stream in
        prefetched = {}
        for i in () if loop_n else (0, 1):
            for nm, dram in (("xqr", xqr_d), ("xk", xk_d), ("xv", xv_d)):
                xt0 = xstage.tile([P, D], f32r, tag=nm)
                nc.sync.dma_start(xt0[:], dram[i * P:(i + 1) * P, :])
                prefetched[(nm, i)] = xt0

        # weights arrive host-pre-rounded to the f32r grid: direct DMA
        w_r = {}
        for name, dram in (("wq", wq_d), ("wk", wk_d), ("wv", wv_d)):
            wr = cpool.tile([P, KC, D], f32r, tag=name + "r")
            nc.sync.dma_start(wr[:], dram.rearrange("(c p) n -> p c n", p=P))
            w_r[name] = wr

        # stage-2-only constants arrive after the weights
        gobo = cpool.tile([P, 2 * D], f32)
        nc.sync.dma_start(gobo[:], gobo_d)
        go_sb = gobo[:, 0:D]
        bo_sb = gobo[:, D:2 * D]
        if has_km:
            km = cpool.tile([P, S], f32)
            nc.sync.dma_start(km[:], km_d)

        # persistent per-batch tensors
        qT = big.tile([P, KC, S], f32r, tag="qT")       # [d_part, dchunk, seq]
        kT = big.tile([P, KC, S], f32r, tag="kT")
        v_sb = big.tile([P, NT, D], f32r, tag="v")      # [kv_part, kvtile, dv]

        # ---- PSUM pools (tpr shared by both phases; stage-1-only pools
        #      closed before stage 2's S/O pools open) ----
        tpr_ps = ctx.enter_context(tc.tile_pool(name="tpr_ps", bufs=2, space="PSUM"))
        s1ctx = ExitStack()
        _pool = s1ctx.enter_context if not loop_n else ctx.enter_context
        tp1_ps = _pool(tc.tile_pool(name="tp1_ps", bufs=1 if loop_n else 3,
                                    space="PSUM"))
        proj_ps = _pool(tc.tile_pool(name="proj_ps", bufs=2 if loop_n else 3,
                                     space="PSUM"))

        if True:

            def proj_tile(x_ap, w, i, kind):
                # batched PE transpose of the four 128x128 input blocks
                xps = tp1_ps.tile([P, D], f32r, tag="tp1")
                for c in range(KC):
                    nc.tensor.transpose(xps[:, c * P:(c + 1) * P],
                                        x_ap[:, c * P:(c + 1) * P], ident_r)
                xt = xt_pool.tile([P, D], f32r, tag="xt")
                nc.scalar.copy(xt[:], xps[:])
                pr = proj_ps.tile([P, D], f32, tag="proj")
                for c in range(KC):
                    nc.tensor.matmul(pr[:], xt[:, c * P:(c + 1) * P], w[:, c, :],
                                     start=(c == 0), stop=(c == KC - 1))
                if kind == "v":
                    nc.scalar.copy(v_sb[:, i, :], pr[:])
                    return
                bn6 = small.tile([P, 6], f32, tag="bn6")
                nc.vector.bn_stats(bn6[:], pr[:])
                agg = small.tile([P, 2], f32, tag="agg")
                nc.vector.bn_aggr(agg[:], bn6[:])
                # rstd = exp(-0.5 * ln(var + eps)) -- stays on ln/exp tables
                lnv = small.tile([P, 1], f32, tag="lnv")
                nc.scalar.activation(lnv[:], agg[:, 1:2], Act.Ln, bias=eps_sb[:])
                rstd = small.tile([P, 1], f32, tag="rstd")
                nc.scalar.activation(rstd[:], lnv[:], Act.Exp, scale=-0.5)
                c1 = small.tile([P, 1], f32, tag="c1")
                nc.vector.tensor_scalar(c1[:], agg[:, 0:1], rstd[:], -1.0,
                                        op0=Alu.mult, op1=Alu.mult)
                y = y_pool.tile([P, D], f32r, tag="y")
                nc.vector.tensor_scalar(y[:], pr[:], rstd[:], c1[:],
                                        op0=Alu.mult, op1=Alu.add)
                # transpose y -> [d, s], fusing gain/bias into eviction
                dstT = qT if kind == "q" else kT
                g = gb["gq" if kind == "q" else "gk"]
                b = gb["bq" if kind == "q" else "bk"]
                yps = tpr_ps.tile([P, D], f32r, tag="tpr")
                for c in range(KC):
                    nc.tensor.transpose(yps[:, c * P:(c + 1) * P],
                                        y[:, c * P:(c + 1) * P], ident_r)
                for c in range(KC):
                    if c < 2:
                        nc.scalar.activation(dstT[:, c, i * P:(i + 1) * P],
                                             yps[:, c * P:(c + 1) * P],
                                             Act.Identity,
                                             bias=b[:, c:c + 1], scale=g[:, c:c + 1])
                    else:
                        nc.vector.tensor_scalar(dstT[:, c, i * P:(i + 1) * P],
                                                yps[:, c * P:(c + 1) * P],
                                                g[:, c:c + 1], b[:, c:c + 1],
                                                op0=Alu.mult, op1=Alu.add)

            def attention(t):
                L = P * (t + 1)
                n_chunks = (L + 511) // 512
                p_sb = p_pool.tile([P, S], f32r, tag="p")
                sums = small.tile([P, KC], f32, tag="sums")
                for c in range(n_chunks):
                    w_cols = min(512, L - c * 512)
                    sc = s_ps.tile([P, 512], f32, tag="s")
                    for kc in range(KC):
                        nc.tensor.matmul(sc[:, :w_cols],
                                         qT[:, kc, t * P:(t + 1) * P],
                                         kT[:, kc, c * 512:c * 512 + w_cols],
                                         start=(kc == 0), stop=(kc == KC - 1))
                    if c * 512 <= t * P < c * 512 + w_cols:
                        off = t * P - c * 512
                        nc.vector.tensor_tensor(sc[:, off:off + P],
                                                sc[:, off:off + P],
                                                tri, op=Alu.add)
                    if has_km:
                        nc.vector.tensor_tensor(sc[:, :w_cols], sc[:, :w_cols],
                                                km[:, c * 512:c * 512 + w_cols],
                                                op=Alu.add)
                    nc.scalar.activation(p_sb[:, c * 512:c * 512 + w_cols],
                                         sc[:, :w_cols], Act.Exp,
                                         accum_out=sums[:, c:c + 1])
                ssum = small.tile([P, 1], f32, tag="ssum")
                nc.vector.tensor_reduce(ssum[:], sums[:, :n_chunks],
                                        axis=mybir.AxisListType.X, op=Alu.add)
                rr = small.tile([P, 1], f32, tag="rr")
                nc.vector.reciprocal(rr[:], ssum[:])

                ops = o_ps.tile([P, D], f32, tag="o")
                for jb in range(0, t + 1, 4):
                    jn = min(4, t + 1 - jb)
                    ptp = tpr_ps.tile([P, D], f32r, tag="tpr")
                    for j in range(jb, jb + jn):
                        nc.tensor.transpose(ptp[:, (j - jb) * P:(j - jb + 1) * P],
                                            p_sb[:, j * P:(j + 1) * P], ident_r)
                    pt_sb = pt_pool.tile([P, D], f32r, tag="pt")
                    if (jb // 4) % 2 == 0:
                        nc.vector.tensor_copy(pt_sb[:, :jn * P], ptp[:, :jn * P])
                    else:
                        nc.scalar.copy(pt_sb[:, :jn * P], ptp[:, :jn * P])
                    for j in range(jb, jb + jn):
                        nc.tensor.matmul(ops[:],
                                         pt_sb[:, (j - jb) * P:(j - jb + 1) * P],
                                         v_sb[:, j, :],
                                         start=(j == 0), stop=(j == t))

                # out = LN(o / rowsum + xq) * go + bo
                xres = z_pool.tile([P, D], f32, tag="xres")
                nc.sync.dma_start(xres[:], xq_d[t * P:(t + 1) * P, :])
                z = z_pool.tile([P, D], f32, tag="z")
                nc.vector.scalar_tensor_tensor(z[:], ops[:], rr[:], xres[:],
                                               op0=Alu.mult, op1=Alu.add)
                bn6 = small.tile([P, 6], f32, tag="bn6")
                nc.vector.bn_stats(bn6[:], z[:])
                agg = small.tile([P, 2], f32, tag="agg")
                nc.vector.bn_aggr(agg[:], bn6[:])
                lnv = small.tile([P, 1], f32, tag="lnv")
                nc.scalar.activation(lnv[:], agg[:, 1:2], Act.Ln, bias=eps_sb[:])
                rstd = small.tile([P, 1], f32, tag="rstd")
                nc.scalar.activation(rstd[:], lnv[:], Act.Exp, scale=-0.5)
                c1 = small.tile([P, 1], f32, tag="c1")
                nc.vector.tensor_scalar(c1[:], agg[:, 0:1], rstd[:], -1.0,
                                        op0=Alu.mult, op1=Alu.mult)
                w1 = z_pool.tile([P, D], f32, tag="w1")
                nc.vector.tensor_scalar(w1[:], z[:], rstd[:], c1[:],
                                        op0=Alu.mult, op1=Alu.add)
                o_sb = z_pool.tile([P, D], f32, tag="osb")
                ttev = nc.vector if t == NT - 1 else nc.gpsimd
                ttev.tensor_tensor(o_sb[:], w1[:], go_sb, op=Alu.mult)
                ttev.tensor_tensor(o_sb[:], o_sb[:], bo_sb, op=Alu.add)
                nc.sync.dma_start(out_d[t * P:(t + 1) * P, :], o_sb[:])

            loop_cm = tc.For_i(0, loop_n, 1) if loop_n else None
            if loop_cm is not None:
                loop_cm.__enter__()
            for i in range(NT):
                for nm, dram, wkey, kind in (("xqr", xqr_d, "wq", "q"),
                                             ("xk", xk_d, "wk", "k"),
                                             ("xv", xv_d, "wv", "v")):
                    if (nm, i) in prefetched:
                        xtile = prefetched.pop((nm, i))
                    else:
                        xtile = xstage.tile([P, D], f32r, tag=nm)
                        nc.sync.dma_start(xtile[:], dram[i * P:(i + 1) * P, :])
                    proj_tile(xtile[:], w_r[wkey], i, kind)
            if not loop_n:
                s1ctx.close()
            s_ps = ctx.enter_context(tc.tile_pool(
                name="s_ps", bufs=2 if loop_n else 3, space="PSUM"))
            o_ps = ctx.enter_context(tc.tile_pool(
                name="o_ps", bufs=1 if loop_n else 3, space="PSUM"))
            for t in range(NT):
                attention(t)
            if loop_cm is not None:
                loop_cm.__exit__(None, None, None)

    nc.compile()
    return nc


def _get_nc(has_km=False):
    key = ("nc", bool(has_km))
    if key not in _CACHE:
        _CACHE[key] = _build(has_km)
    return _CACHE[key]


def _fallback(vals, keys, ques, causal_mask, key_mask, Wv, Wk, Wq,
              ln_k_g, ln_k_b, ln_q_g, ln_q_b, ln_o_g, ln_o_b):
    # numpy reference path; only used if causal_mask is not the standard
    # triangular pattern this kernel is specialized for.
    def ln(x, g, b):
        mu = x.mean(-1, keepdims=True)
        var = ((x - mu) ** 2).mean(-1, keepdims=True)
        return (x - mu) / np.sqrt(var + EPS) * g + b

    x64 = np.float64
    v = vals.astype(x64) @ Wv.astype(x64)
    k = ln(keys.astype(x64) @ Wk.astype(x64), ln_k_g, ln_k_b)
    q = ln(ques.astype(x64) @ Wq.astype(x64), ln_q_g, ln_q_b)
    a = np.einsum("bqd,bkd->bqk", q, k) / math.sqrt(D)
    a = np.where(causal_mask[None], -np.inf, a)
    a = np.where(key_mask[:, None, :], -np.inf, a)
    a = a - a.max(-1, keepdims=True)
    p = np.exp(a)
    p /= p.sum(-1, keepdims=True)
    o = np.einsum("bqk,bkd->bqd", p, v)
    return ln(o + ques.astype(x64), ln_o_g, ln_o_b).astype(np.float32)


def _get_runner(has_km):
    """Build (once) a cached sharded-jit executor for the compiled module.

    run_bass_kernel_spmd re-creates and re-traces its jitted body on every
    call; caching the jit object makes repeat kernel() calls cheap.
    """
    key = ("runner", bool(has_km))
    if key in _CACHE:
        return _CACHE[key]

    import jax
    import numpy as _np
    from jax.sharding import Mesh, PartitionSpec
    from jax.experimental.shard_map import shard_map
    from concourse import mybir
    from concourse.bass2jax import (_bass_exec_p, install_neuronx_cc_hook,
                                    partition_id_tensor)

    install_neuronx_cc_hook()
    nc = _get_nc(has_km)

    pname = nc.partition_id_tensor.name if nc.partition_id_tensor else None
    in_names, out_names, out_avals, zero_outs = [], [], [], []
    for alloc in nc.m.functions[0].allocations:
        if not isinstance(alloc, mybir.MemoryLocationSet):
            continue
        name = alloc.memorylocations[0].name
        if alloc.kind == "ExternalInput":
            if name != pname:
                in_names.append(name)
        elif alloc.kind == "ExternalOutput":
            shape = tuple(alloc.tensor_shape)
            dtype = mybir.dt.np(alloc.dtype)
            out_names.append(name)
            out_avals.append(jax.core.ShapedArray(shape, dtype))
            zero_outs.append(_np.zeros((B * shape[0], *shape[1:]), dtype))
    n_params = len(in_names)
    all_in = in_names + out_names
    if pname is not None:
        all_in = all_in + [pname]

    def _body(*args):
        operands = list(args)
        if pname is not None:
            operands.append(partition_id_tensor())
        outs = _bass_exec_p.bind(
            *operands,
            out_avals=tuple(out_avals),
            in_names=tuple(all_in),
            out_names=tuple(out_names),
            lowering_input_output_aliases=(),
            sim_require_finite=True,
            sim_require_nnan=True,
            nc=nc,
        )
        return tuple(outs)

    devices = jax.devices()[:B]
    mesh = Mesh(np.asarray(devices), ("core",))
    donate = tuple(range(n_params, n_params + len(out_names)))
    sharded = jax.jit(
        shard_map(_body, mesh=mesh,
                  in_specs=(PartitionSpec("core"),) * (n_params + len(out_names)),
                  out_specs=(PartitionSpec("core"),) * len(out_names),
                  check_rep=False),
        donate_argnums=donate, keep_unused=True)

    def run(concat_by_name):
        args = [concat_by_name[n] for n in in_names] + list(zero_outs)
        out_arrs = sharded(*args)
        return {n: _np.asarray(out_arrs[i]).reshape(B, *out_avals[i].shape)
                for i, n in enumerate(out_names)}

    _CACHE[key] = run
    return run


def kernel(vals, keys, ques, causal_mask, key_mask, Wv, Wk, Wq,
           ln_k_g, ln_k_b, ln_q_g, ln_q_b, ln_o_g, ln_o_b):
    causal_mask = np.asarray(causal_mask)
    key_mask = np.asarray(key_mask)
    if not np.array_equal(causal_mask, np.triu(np.ones((S, S), bool), k=1)):
        return _fallback(vals, keys, ques, causal_mask, key_mask, Wv, Wk, Wq,
                         ln_k_g, ln_k_b, ln_q_g, ln_q_b, ln_o_g, ln_o_b)

    has_km = bool(key_mask.any())
    run = _get_runner(has_km)

    f = np.float32
    scale = f(1.0 / math.sqrt(D))

    def chunked(v):
        # [D] vector -> [P, KC] chunk layout (column c = chunk c)
        return np.ascontiguousarray(np.asarray(v, f).reshape(KC, P).T)

    gbv = np.concatenate([chunked(np.asarray(ln_q_g, f) * scale),
                          chunked(np.asarray(ln_q_b, f) * scale),
                          chunked(ln_k_g), chunked(ln_k_b)], axis=1)  # [P, 16]
    gobo = np.broadcast_to(
        np.concatenate([np.asarray(ln_o_g, f), np.asarray(ln_o_b, f)]),
        (P, 2 * D)).copy()
    tri = np.where(causal_mask[:P, :P], NEG, f(0)).astype(f)
    ident = np.eye(P, dtype=f)
    wq = _round_f32r(np.ascontiguousarray(Wq, f))
    wk = _round_f32r(np.ascontiguousarray(Wk, f))
    wv = _round_f32r(np.ascontiguousarray(Wv, f))
    xq = np.ascontiguousarray(ques, f).reshape(B * S, D)

    def rep(a):
        # replicate a shared param: concat along axis 0 for shard_map
        return np.concatenate([a] * B, axis=0)

    km_rows = np.where(key_mask, NEG, f(0)).astype(f)          # [B, S]
    km_cat = np.repeat(km_rows, P, axis=0)                      # [B*P, S]
    concat = {
        "xq": xq,
        "xqr": _round_f32r(xq),
        "xk": _round_f32r(np.ascontiguousarray(keys, f).reshape(B * S, D)),
        "xv": _round_f32r(np.ascontiguousarray(vals, f).reshape(B * S, D)),
        "wq": rep(wq), "wk": rep(wk), "wv": rep(wv),
        "gbv": rep(gbv), "gobo": rep(gobo),
        "tri": rep(tri), "km": km_cat, "identr": rep(ident),
    }
    out = run(concat)["out"]                                    # [B, S, D]
    return out
